# revision 18
# baseline (speedup 1.0000x reference)
"""LoRA linear kernel for Trainium2 (8 NeuronCores, SPMD data-parallel).

Computes y = x @ (B @ A)^T for
    x: [4, 2048, 4096] f32, B: [4096, 16] f32, A: [16, 4096] f32.

Strategy: never materialize W = B @ A.  Factor as t = x @ A^T (rank 16)
then y = t @ B^T.  Tokens (4*2048 = 8192) are sharded across 8 cores
(1024 tokens each); A and B are replicated.

The kernel is HBM-DMA-bound, so both streams are bf16 on the wire:
  - x is cast to bf16 on the host (halves the read; the rank-4096
    contraction keeps the rounding error ~1e-3, inside the 2e-2 gate);
  - y is written as bf16 and upcast to f32 on the host (halves the
    write; adds <=2^-9 relative error).
Per-core traffic is 8.4MB in + 8.4MB out vs 33.6MB for all-f32.

Two DMA rings overlap: x chunks ride the Sync HWDGE ring, at/bt and the
y stores ride the Scalar (ACT) ring, so write drain overlaps the read
phase and the DMA engines stay busy end-to-end.

PE schedule (2 groups of 512 tokens):
  mm1: t^T[16, 512] = sum_ko A^T[ko] (lhsT [128,16]) . x^T[ko] (rhs [128,512])
       accumulated in 8-matmul octets as each 1MB chunk lands
  mm2: y[tok128, o] = t^T[:, c*128:...] (lhsT [16,128]) . B^T (rhs [16,512])
Octets are read-gated (a chunk arrives every ~2.7us, the warm PE eats
one in ~1.7us), so: junk matmuls on a memset tile pre-warm the HAM
clock gate during the prologue and pad group 0's octet waits; group 1's
octets interleave with group 0's mm2 chunks; the last mm2 chunk of
group 0 doubles as the seam pad that covers the tT1 copy latency.  Any
PE idle >~2us lets HAM re-throttle the PE from 2.4 to 1.2 GHz, which
measured as the single biggest source of lost time.

PSUM evacuation uses single-bank copies only (the DVE fast path:
341ns/bank; anything crossing a bank boundary measured 1.2us), DVE 6 :
ACT 2 per chunk over a 6-buf psum pool.
"""

import sys

import numpy as np

if "/opt/trn_rl_repo" not in sys.path:
    sys.path.insert(0, "/opt/trn_rl_repo")

# Problem shape (hardcoded per contract)
BATCH = 4
SEQ = 2048
D = 4096          # in_features == out_features
R = 16            # lora rank
NCORES = 8
NTOK = BATCH * SEQ            # 8192 tokens total
TOK = NTOK // NCORES          # 1024 tokens per core
P = 128                       # partitions
KO = D // P                   # 32 feature chunks
TB = 512                      # tokens per mm1 group (matmul free dim)
NG = TOK // TB                # 2 groups per core
NCHG = 4                      # x DMA chunks per group (1MB each)
KOC = KO // NCHG              # 8 ko-slices per chunk
NB = 512                      # matmul free dim for mm2 (psum bank limit)

# Module-level knobs for test.py (harness never touches these)
TRACE = False
LAST_RESULTS = None

_nc_cache = None


def _build_program():
    from concourse import bacc, mybir, tile

    # Bacc (not raw Bass): its finalize() runs generate_event_semaphores,
    # which splits multi-sem waits to satisfy TRN2's 1-wait-per-instruction
    # hardware constraint (walrus rejects >1 otherwise).
    nc = bacc.Bacc(
        "TRN2", target_bir_lowering=False, debug=False, num_devices=NCORES
    )

    f32 = mybir.dt.float32
    bf16 = mybir.dt.bfloat16

    xt = nc.dram_tensor("xt", [NG, NCHG, P, KOC, TB], bf16, kind="ExternalInput")
    at = nc.dram_tensor("at", [P, KO, R], bf16, kind="ExternalInput")
    bt = nc.dram_tensor("bt", [R, D], bf16, kind="ExternalInput")
    y = nc.dram_tensor("y", [TOK, D], bf16, kind="ExternalOutput")

    with tile.TileContext(nc) as tc:
        with (
            tc.tile_pool(name="consts", bufs=1) as consts,
            tc.tile_pool(name="xin", bufs=NG * NCHG) as xin,
            tc.tile_pool(name="tbuf", bufs=2) as tbuf,
            tc.tile_pool(name="yout", bufs=8) as yout,
            tc.tile_pool(name="pt", bufs=2, space="PSUM") as pt_pool,
            tc.tile_pool(name="py", bufs=6, space="PSUM") as py_pool,
        ):
            # consts ride the ACT HWDGE ring so the Sync ring starts
            # streaming x immediately
            at_s = consts.tile([P, KO, R], bf16)
            nc.scalar.dma_start(at_s[:], at[:])
            bt_s = consts.tile([R, D], bf16)
            nc.scalar.dma_start(bt_s[:], bt[:])

            # HAM pre-warm: junk matmuls on a memset tile, gated only on
            # the memset, so the PE clock ramps to 8/8 (2.4 GHz) during the
            # DMA prologue instead of during mm1 of the first group.
            junk = consts.tile([P, NB], bf16)
            nc.gpsimd.memset(junk[:], 0.0)

            def pe_warm(n):
                for _ in range(n):
                    warm = py_pool.tile([P, NB], f32, tag="psum_y")
                    nc.tensor.matmul(
                        warm[:], junk[:, :P], junk[:],
                        start=True, stop=True, skip_group_check=True,
                    )

            pe_warm(8)
            tc.no_sync_barrier()

            def mm1_octet(g, c4, psum_t):
                # one 1MB fully-contiguous x chunk -> 8 accumulating matmuls
                xt_tile = xin.tile([P, KOC, TB], bf16, tag="xt")
                nc.sync.dma_start(xt_tile[:], xt[g, c4])
                for j in range(KOC):
                    ko = c4 * KOC + j
                    nc.tensor.matmul(
                        psum_t[:],
                        at_s[:, ko, :],
                        xt_tile[:, j, :],
                        start=(ko == 0),
                        stop=(ko == KO - 1),
                        skip_group_check=True,
                    )

            def make_tT(psum_t):
                # DVE copy psum f32 -> bf16 for the mm2 stationary operand
                tT = tbuf.tile([R, TB], bf16)
                nc.vector.tensor_copy(tT[:], psum_t[:])
                return tT

            def pe_warm_t(n):
                # tail-phase pads draw on the pt pool (psum_t0/1 are dead
                # by then) so they never contend with mm2's psum_y bufs
                for _ in range(n):
                    warm = pt_pool.tile([R, TB], f32, tag="psum_t")
                    nc.tensor.matmul(
                        warm[:], junk[:, :R], junk[:],
                        start=True, stop=True, skip_group_check=True,
                    )

            def mm2_chunk(g, c, tT, tail=False):
                y_row = yout.tile([P, D], bf16)
                for n in range(D // NB):
                    psum_y = py_pool.tile([P, NB], f32, tag="psum_y")
                    nc.tensor.matmul(
                        psum_y[:],
                        tT[:, c * P : (c + 1) * P],
                        bt_s[:, n * NB : (n + 1) * NB],
                        start=True,
                        stop=True,
                        skip_group_check=True,
                    )
                    # Single-bank PSUM evacuation, DVE 6 : ACT 2
                    if n % 3 == 2:
                        nc.scalar.copy(y_row[:, n * NB : (n + 1) * NB], psum_y[:])
                    else:
                        nc.vector.tensor_copy(y_row[:, n * NB : (n + 1) * NB], psum_y[:])
                    if tail and n % 4 == 3:
                        # copy-bound phase: the PE would idle ~0.6us per MM
                        # on psum reuse, enough for HAM to re-throttle the
                        # clock; junk matmuls keep it busy and warm
                        pe_warm_t(1)
                row0 = g * TB + c * P
                if tail:
                    # reads are done by now: the idle Sync ring takes the
                    # store so ACT only runs copies
                    nc.sync.dma_start(y[row0 : row0 + P, :], y_row[:])
                else:
                    # scalar-engine HWDGE ring: write drain overlaps reads
                    nc.scalar.dma_start(y[row0 : row0 + P, :], y_row[:])

            # Software-pipelined schedule (see module docstring).
            psum_t0 = pt_pool.tile([R, TB], f32, tag="psum_t")
            for c4 in range(NCHG):
                pe_warm(4)
                mm1_octet(0, c4, psum_t0)
            tT0 = make_tT(psum_t0)
            pe_warm(2)

            psum_t1 = pt_pool.tile([R, TB], f32, tag="psum_t")
            for c4 in range(NCHG):
                if c4 < 3:
                    mm2_chunk(0, c4, tT0)
                mm1_octet(1, c4, psum_t1)
            tT1 = make_tT(psum_t1)
            mm2_chunk(0, 3, tT0, tail=True)

            for c in range(NCHG):
                mm2_chunk(1, c, tT1, tail=True)

    nc.finalize()
    return nc


def kernel(x, lora_matrix_B, lora_matrix_A):
    global _nc_cache, LAST_RESULTS
    import ml_dtypes
    from concourse.bass_utils import run_bass_kernel_spmd

    if _nc_cache is None:
        _nc_cache = _build_program()
    nc = _nc_cache

    bf16 = ml_dtypes.bfloat16
    x_flat = np.asarray(x, dtype=np.float32).reshape(NTOK, D).astype(bf16)
    A = np.asarray(lora_matrix_A, dtype=np.float32).astype(bf16)
    B = np.asarray(lora_matrix_B, dtype=np.float32).astype(bf16)

    # at[p, ko, j] = A[j, ko*128 + p];  bt[j, o] = B[o, j]
    at_prep = np.ascontiguousarray(A.reshape(R, KO, P).transpose(2, 1, 0))
    bt_prep = np.ascontiguousarray(B.T)

    in_maps = []
    for core in range(NCORES):
        xc = x_flat[core * TOK : (core + 1) * TOK, :]
        # xt[g, c4, p, j, t] = xc[g*512 + t, (c4*8 + j)*128 + p]
        xt_prep = np.ascontiguousarray(
            xc.reshape(NG, TB, NCHG, KOC, P).transpose(0, 2, 4, 3, 1)
        )
        in_maps.append({"xt": xt_prep, "at": at_prep, "bt": bt_prep})

    res = run_bass_kernel_spmd(
        nc, in_maps, core_ids=list(range(NCORES)), trace=TRACE
    )
    LAST_RESULTS = res

    y = np.concatenate([res.results[c]["y"] for c in range(NCORES)], axis=0)
    return y.reshape(BATCH, SEQ, D).astype(np.float32)


# revision 20
# speedup vs baseline: 1.2532x; 1.2532x over previous
"""LoRA linear kernel for Trainium2 (8 NeuronCores, SPMD data-parallel).

Computes y = x @ (B @ A)^T for
    x: [4, 2048, 4096] f32, B: [4096, 16] f32, A: [16, 4096] f32.

Strategy: never materialize W = B @ A.  Factor as t = x @ A^T (rank 16)
then y = t @ B^T.  Tokens (4*2048 = 8192) are sharded across 8 cores
(1024 tokens each); A and B are replicated.

The kernel is HBM-DMA-bound, so both streams are bf16 on the wire:
  - x is cast to bf16 on the host (halves the read; the rank-4096
    contraction keeps the rounding error ~1e-3, inside the 2e-2 gate);
  - y is written as bf16 and upcast to f32 on the host (halves the
    write; adds <=2^-9 relative error).
Per-core traffic is 8.4MB in + 8.4MB out vs 33.6MB for all-f32.

Two DMA rings overlap: x chunks ride the Sync HWDGE ring, at/bt and the
y stores ride the Scalar (ACT) ring, so write drain overlaps the read
phase and the DMA engines stay busy end-to-end.

PE schedule (2 groups of 512 tokens):
  mm1: t^T[16, 512] = sum_ko A^T[ko] (lhsT [128,16]) . x^T[ko] (rhs [128,512])
       accumulated in 8-matmul octets as each 1MB chunk lands
  mm2: y[tok128, o] = t^T[:, c*128:...] (lhsT [16,128]) . B^T (rhs [16,512])
Octets are read-gated (a chunk arrives every ~2.7us, the warm PE eats
one in ~1.7us), so: junk matmuls on a memset tile pre-warm the HAM
clock gate during the prologue and pad group 0's octet waits; group 1's
octets interleave with group 0's mm2 chunks; the last mm2 chunk of
group 0 doubles as the seam pad that covers the tT1 copy latency.  Any
PE idle >~2us lets HAM re-throttle the PE from 2.4 to 1.2 GHz, which
measured as the single biggest source of lost time.

PSUM evacuation uses single-bank copies only (the DVE fast path:
341ns/bank; anything crossing a bank boundary measured 1.2us), DVE 6 :
ACT 2 per chunk over a 6-buf psum pool.
"""

import sys

import numpy as np

if "/opt/trn_rl_repo" not in sys.path:
    sys.path.insert(0, "/opt/trn_rl_repo")

# Problem shape (hardcoded per contract)
BATCH = 4
SEQ = 2048
D = 4096          # in_features == out_features
R = 16            # lora rank
NCORES = 8
NTOK = BATCH * SEQ            # 8192 tokens total
TOK = NTOK // NCORES          # 1024 tokens per core
P = 128                       # partitions
KO = D // P                   # 32 feature chunks
TB = 512                      # tokens per mm1 group (matmul free dim)
NG = TOK // TB                # 2 groups per core
NCHG = 4                      # x DMA chunks per group (1MB each)
KOC = KO // NCHG              # 8 ko-slices per chunk
NB = 512                      # matmul free dim for mm2 (psum bank limit)

# Module-level knobs for test.py (harness never touches these)
TRACE = False
LAST_RESULTS = None

_nc_cache = None


def _build_program():
    from concourse import bacc, mybir, tile

    # Bacc (not raw Bass): its finalize() runs generate_event_semaphores,
    # which splits multi-sem waits to satisfy TRN2's 1-wait-per-instruction
    # hardware constraint (walrus rejects >1 otherwise).
    nc = bacc.Bacc(
        "TRN2", target_bir_lowering=False, debug=False, num_devices=NCORES
    )

    f32 = mybir.dt.float32
    bf16 = mybir.dt.bfloat16

    xt = nc.dram_tensor("xt", [NG, NCHG, P, KOC, TB], bf16, kind="ExternalInput")
    at = nc.dram_tensor("at", [P, KO, R], bf16, kind="ExternalInput")
    bt = nc.dram_tensor("bt", [R, D], bf16, kind="ExternalInput")
    y = nc.dram_tensor("y", [TOK, D], bf16, kind="ExternalOutput")

    with tile.TileContext(nc) as tc:
        with (
            tc.tile_pool(name="consts", bufs=1) as consts,
            tc.tile_pool(name="xin", bufs=NG * NCHG) as xin,
            tc.tile_pool(name="tbuf", bufs=2) as tbuf,
            tc.tile_pool(name="yout", bufs=8) as yout,
            tc.tile_pool(name="pt", bufs=2, space="PSUM") as pt_pool,
            tc.tile_pool(name="py", bufs=6, space="PSUM") as py_pool,
        ):
            # consts ride the ACT HWDGE ring so the Sync ring starts
            # streaming x immediately
            at_s = consts.tile([P, KO, R], bf16)
            nc.scalar.dma_start(at_s[:], at[:])
            bt_s = consts.tile([R, D], bf16)
            nc.scalar.dma_start(bt_s[:], bt[:])

            # HAM pre-warm: junk matmuls on a memset tile, gated only on
            # the memset, so the PE clock ramps to 8/8 (2.4 GHz) during the
            # DMA prologue instead of during mm1 of the first group.
            junk = consts.tile([P, NB], bf16)
            nc.gpsimd.memset(junk[:], 0.0)

            def pe_warm(n):
                for _ in range(n):
                    warm = py_pool.tile([P, NB], f32, tag="psum_y")
                    nc.tensor.matmul(
                        warm[:], junk[:, :P], junk[:],
                        start=True, stop=True, skip_group_check=True,
                    )

            pe_warm(8)
            tc.no_sync_barrier()

            def mm1_octet(g, c4, psum_t):
                # one 1MB fully-contiguous x chunk -> 8 accumulating matmuls
                xt_tile = xin.tile([P, KOC, TB], bf16, tag="xt")
                nc.sync.dma_start(xt_tile[:], xt[g, c4])
                for j in range(KOC):
                    ko = c4 * KOC + j
                    nc.tensor.matmul(
                        psum_t[:],
                        at_s[:, ko, :],
                        xt_tile[:, j, :],
                        start=(ko == 0),
                        stop=(ko == KO - 1),
                        skip_group_check=True,
                    )

            def make_tT(psum_t):
                # DVE copy psum f32 -> bf16 for the mm2 stationary operand
                tT = tbuf.tile([R, TB], bf16)
                nc.vector.tensor_copy(tT[:], psum_t[:])
                return tT

            def pe_warm_t(n):
                # tail-phase pads draw on the pt pool (psum_t0/1 are dead
                # by then) so they never contend with mm2's psum_y bufs
                for _ in range(n):
                    warm = pt_pool.tile([R, TB], f32, tag="psum_t")
                    nc.tensor.matmul(
                        warm[:], junk[:, :R], junk[:],
                        start=True, stop=True, skip_group_check=True,
                    )

            def mm2_chunk(g, c, tT, tail=False):
                y_row = yout.tile([P, D], bf16)
                for n in range(D // NB):
                    psum_y = py_pool.tile([P, NB], f32, tag="psum_y")
                    nc.tensor.matmul(
                        psum_y[:],
                        tT[:, c * P : (c + 1) * P],
                        bt_s[:, n * NB : (n + 1) * NB],
                        start=True,
                        stop=True,
                        skip_group_check=True,
                    )
                    # Single-bank PSUM evacuation, DVE 6 : ACT 2
                    if n % 3 == 2:
                        nc.scalar.copy(y_row[:, n * NB : (n + 1) * NB], psum_y[:])
                    else:
                        nc.vector.tensor_copy(y_row[:, n * NB : (n + 1) * NB], psum_y[:])
                    if tail and n % 4 == 3:
                        # copy-bound phase: the PE would idle ~0.6us per MM
                        # on psum reuse, enough for HAM to re-throttle the
                        # clock; junk matmuls keep it busy and warm
                        pe_warm_t(1)
                row0 = g * TB + c * P
                # scalar-engine HWDGE ring: write drain overlaps the reads
                nc.scalar.dma_start(y[row0 : row0 + P, :], y_row[:])

            # Software-pipelined schedule (see module docstring).
            psum_t0 = pt_pool.tile([R, TB], f32, tag="psum_t")
            for c4 in range(NCHG):
                pe_warm(4)
                mm1_octet(0, c4, psum_t0)
            tT0 = make_tT(psum_t0)
            pe_warm(2)

            psum_t1 = pt_pool.tile([R, TB], f32, tag="psum_t")
            for c4 in range(NCHG):
                mm1_octet(1, c4, psum_t1)
                if c4 < 3:
                    mm2_chunk(0, c4, tT0)
            tT1 = make_tT(psum_t1)
            mm2_chunk(0, 3, tT0, tail=True)

            for c in range(NCHG):
                mm2_chunk(1, c, tT1, tail=True)

    nc.finalize()
    return nc


def kernel(x, lora_matrix_B, lora_matrix_A):
    global _nc_cache, LAST_RESULTS
    import ml_dtypes
    from concourse.bass_utils import run_bass_kernel_spmd

    if _nc_cache is None:
        _nc_cache = _build_program()
    nc = _nc_cache

    bf16 = ml_dtypes.bfloat16
    x_flat = np.asarray(x, dtype=np.float32).reshape(NTOK, D).astype(bf16)
    A = np.asarray(lora_matrix_A, dtype=np.float32).astype(bf16)
    B = np.asarray(lora_matrix_B, dtype=np.float32).astype(bf16)

    # at[p, ko, j] = A[j, ko*128 + p];  bt[j, o] = B[o, j]
    at_prep = np.ascontiguousarray(A.reshape(R, KO, P).transpose(2, 1, 0))
    bt_prep = np.ascontiguousarray(B.T)

    in_maps = []
    for core in range(NCORES):
        xc = x_flat[core * TOK : (core + 1) * TOK, :]
        # xt[g, c4, p, j, t] = xc[g*512 + t, (c4*8 + j)*128 + p]
        xt_prep = np.ascontiguousarray(
            xc.reshape(NG, TB, NCHG, KOC, P).transpose(0, 2, 4, 3, 1)
        )
        in_maps.append({"xt": xt_prep, "at": at_prep, "bt": bt_prep})

    res = run_bass_kernel_spmd(
        nc, in_maps, core_ids=list(range(NCORES)), trace=TRACE
    )
    LAST_RESULTS = res

    y = np.concatenate([res.results[c]["y"] for c in range(NCORES)], axis=0)
    return y.reshape(BATCH, SEQ, D).astype(np.float32)
